# revision 4
# baseline (speedup 1.0000x reference)
"""DNC-style LSTM-with-memory-read kernel for 8 Trainium2 NeuronCores.

Math summary (derived from the reference):
  The torch-faithful [R,B,M]->[B,R*M] view means row b' of the new read
  vector is concat_k read[(4*b'+k) mod B]. Since read = h @ mem_sm.T and
  rv only enters the LSTM through W_ih's rv columns (W_rv), the rv
  contribution to the gates collapses to a "mix" term:
      gates[b'] += sum_k h[4*u(b')+k] @ G_k,   u(b') = b' mod 256
  with G_k = mem_sm.T @ W_rv[:, k*M:(k+1)*M].T precomputed on host.
  The final fc layer is linear in h and read, and the output is a mean
  over time, so it reduces to a function of hsum = sum_t h_t — computed
  on host from each core's hsum shard.

Distribution: batch is sharded contiguously over 8 cores (128 rows each).
The mix term couples rows across shards (provably fully-mixing within 5
steps), so each step AllGathers hT (64KB/core). Each core then reads the
half of the gathered buffer its parity needs via a partition_id-derived
register offset, and computes its gates with 7 matmuls into one PSUM bank:
2 x-projection (prefetched), 1 W_hh, 4 mix (strided lhsT over gathered hT).
"""

import sys

if '/opt/trn_rl_repo' not in sys.path:
    sys.path.insert(0, '/opt/trn_rl_repo')

import numpy as np

B, T, D_IN = 1024, 128, 256
H = 128
M = 128
W = 128
R = 4
OUT = 2
NCORES = 8
RL = B // NCORES  # 128 local rows per core

_PROGRAM_CACHE = {}


def build_program(t_steps=T, bf16_mix=False):
    """Build (and compile) the SPMD Bass program for t_steps timesteps.

    bf16_mix: carry the AllGathered hT and the mix matmuls in bf16.
    The mix term is ~4% of the gate magnitude, so bf16 there perturbs
    gates by ~2e-4 relative — negligible vs the fp32 recurrence.
    """
    import concourse.bass as bass
    import concourse.bacc as bacc
    import concourse.mybir as mybir
    import concourse.tile as tile
    from concourse.masks import make_identity

    f32 = mybir.dt.float32
    bf16 = mybir.dt.bfloat16
    mixdt = bf16 if bf16_mix else f32
    AF = mybir.ActivationFunctionType

    nc = bacc.Bacc(
        "TRN2",
        target_bir_lowering=False,
        debug=False,
        enable_asserts=False,
        num_devices=NCORES,
    )

    # Inputs (host-side layouts are pre-arranged for partition-major DMA)
    xT = nc.dram_tensor("xT", [t_steps, 128, 2, RL], f32, kind="ExternalInput")
    wxT = nc.dram_tensor("wxT", [128, 2, 512], f32, kind="ExternalInput")
    whhT = nc.dram_tensor("whhT", [128, 512], f32, kind="ExternalInput")
    gmat = nc.dram_tensor("gmat", [128, 4, 512], mixdt, kind="ExternalInput")
    biasb = nc.dram_tensor("biasb", [128, 512], f32, kind="ExternalInput")
    bias1b = nc.dram_tensor("bias1b", [128, 512], f32, kind="ExternalInput")
    hsum_out = nc.dram_tensor("hsum_out", [RL, H], f32, kind="ExternalOutput")

    with tile.TileContext(nc) as tc:
        with (
            tc.tile_pool(name="const", bufs=1) as cpool,
            tc.tile_pool(name="xin", bufs=4) as xpool,
            tc.tile_pool(name="work", bufs=2) as wpool,
            tc.tile_pool(name="gt", bufs=3) as gtpool,
            tc.tile_pool(name="psg", bufs=4, space="PSUM") as psg,
            tc.tile_pool(name="pst", bufs=2, space="PSUM") as pst,
            tc.tile_pool(name="dram", bufs=2, space="DRAM") as dpool,
        ):
            ident = cpool.tile([128, 128], f32)
            make_identity(nc, ident)
            wx_sb = cpool.tile([128, 2, 512], f32)
            nc.sync.dma_start(wx_sb[:], wxT[:])
            whh_sb = cpool.tile([128, 512], f32)
            nc.sync.dma_start(whh_sb[:], whhT[:])
            g_sb = cpool.tile([128, 4, 512], mixdt)
            nc.sync.dma_start(g_sb[:], gmat[:])
            bb_sb = cpool.tile([128, 512], f32)
            nc.sync.dma_start(bb_sb[:], biasb[:])
            b1_sb = cpool.tile([128, 512], f32)
            nc.sync.dma_start(b1_sb[:], bias1b[:])
            hsum = cpool.tile([RL, H], f32)
            nc.vector.memset(hsum[:], 0.0)

            # which half of the gathered hT this core's mix needs
            pid = nc.sync.partition_id()
            roff = nc.sync.compute_val((pid % 2) * 4)

            hT_sb = None
            hgat = None
            c_prev = None

            for t in range(1, t_steps + 1):
                # ---- x-projection for step t (independent of recurrence,
                #      prefetches ahead and fills PE gaps during AllGather)
                xt = xpool.tile([128, 2, RL], f32, tag="xt")
                nc.sync.dma_start(xt[:], xT[t - 1])
                psum_g = psg.tile([RL, 512], f32, tag="g")
                for c_ in range(2):
                    nc.tensor.matmul(
                        psum_g[:],
                        xt[:, c_, :],
                        wx_sb[:, c_, :],
                        start=(c_ == 0),
                        stop=(t == 1 and c_ == 1),
                    )
                # ---- recurrent terms (use h_{t-1} local + gathered)
                if t >= 2:
                    nc.tensor.matmul(
                        psum_g[:], hT_sb[:], whh_sb[:], start=False, stop=False
                    )
                    hflat = hgat.rearrange("p r f -> p (r f)")
                    for k in range(4):
                        nc.tensor.matmul(
                            psum_g[:],
                            hflat[:, k::4],
                            g_sb[:, k, :],
                            start=False,
                            stop=(k == 3),
                        )
                bias_t = b1_sb if t == 1 else bb_sb
                gates = gtpool.tile([RL, 512], f32, tag="gates")
                for gi in range(4):
                    sl = slice(128 * gi, 128 * (gi + 1))
                    nc.vector.tensor_add(gates[:, sl], psum_g[:, sl], bias_t[:, sl])
                acti = wpool.tile([RL, 128], f32, tag="acti")
                actf = wpool.tile([RL, 128], f32, tag="actf")
                actg = wpool.tile([RL, 128], f32, tag="actg")
                acto = wpool.tile([RL, 128], f32, tag="acto")
                nc.scalar.activation(acti[:], gates[:, 0:128], AF.Sigmoid)
                nc.scalar.activation(actf[:], gates[:, 128:256], AF.Sigmoid)
                nc.scalar.activation(actg[:], gates[:, 256:384], AF.Tanh)
                nc.scalar.activation(acto[:], gates[:, 384:512], AF.Sigmoid)
                t2 = wpool.tile([RL, 128], f32, tag="t2")
                nc.vector.tensor_mul(t2[:], acti[:], actg[:])
                c_new = wpool.tile([RL, 128], f32, tag="c")
                if t == 1:
                    nc.vector.tensor_copy(c_new[:], t2[:])
                else:
                    t1 = wpool.tile([RL, 128], f32, tag="t1")
                    nc.vector.tensor_mul(t1[:], actf[:], c_prev[:])
                    nc.vector.tensor_add(c_new[:], t1[:], t2[:])
                c_prev = c_new
                tch = wpool.tile([RL, 128], f32, tag="tch")
                nc.scalar.activation(tch[:], c_new[:], AF.Tanh)
                h = wpool.tile([RL, 128], f32, tag="h")
                nc.vector.tensor_mul(h[:], acto[:], tch[:])
                nc.vector.tensor_add(hsum[:], hsum[:], h[:])
                if t < t_steps:
                    ps_hT = pst.tile([128, RL], f32, tag="htr")
                    nc.tensor.transpose(ps_hT[:], h[:], ident[:])
                    hT_sb = wpool.tile([128, RL], f32, tag="hT")
                    nc.scalar.copy(hT_sb[:], ps_hT[:])
                    if bf16_mix:
                        hTb = wpool.tile([128, RL], bf16, tag="hTb")
                        nc.vector.tensor_copy(hTb[:], ps_hT[:])
                        ag_src = hTb
                    else:
                        ag_src = hT_sb
                    ag_in = dpool.tile([128, RL], mixdt, tag="agin")
                    nc.sync.dma_start(ag_in[:], ag_src[:])
                    ag_out = dpool.tile(
                        [NCORES * 128, RL], mixdt, tag="agout", addr_space="Shared"
                    )
                    nc.gpsimd.collective_compute(
                        "AllGather",
                        mybir.AluOpType.bypass,
                        replica_groups=[list(range(NCORES))],
                        ins=[ag_in[:]],
                        outs=[ag_out[:]],
                    )
                    hgat = gtpool.tile([128, 4, RL], mixdt, tag="hgat")
                    src = ag_out.rearrange("(r p) f -> p r f", p=128)[
                        :, bass.ds(roff, 4), :
                    ]
                    nc.sync.dma_start(hgat[:], src)

            nc.sync.dma_start(hsum_out[:], hsum[:])

    nc.compile()
    return nc


def host_prep(inputs, t_steps=T, bf16_mix=False):
    """Host-side parameter folding + per-core input maps."""
    x = np.asarray(inputs["x"], dtype=np.float32)
    memory = np.asarray(inputs["memory"], dtype=np.float64)
    rv0 = np.asarray(inputs["read_vectors0"], dtype=np.float64)
    W_ih = np.asarray(inputs["W_ih"], dtype=np.float64)
    W_hh = np.asarray(inputs["W_hh"], dtype=np.float64)
    b_ih = np.asarray(inputs["b_ih"], dtype=np.float64)
    b_hh = np.asarray(inputs["b_hh"], dtype=np.float64)

    # softmax over memory slots (dim 0)
    mm = memory - memory.max(axis=0, keepdims=True)
    e = np.exp(mm)
    mem_sm = e / e.sum(axis=0, keepdims=True)  # [M, W]

    W_x = W_ih[:, :D_IN]          # [4H, D_IN]
    W_rv = W_ih[:, D_IN:]         # [4H, R*W]
    bias = b_ih + b_hh            # [4H]
    bias1 = bias + rv0.reshape(R * W) @ W_rv.T

    # G[128k + j, c] = (mem_sm.T @ W_rv[:, kM:(k+1)M].T)[j, c]
    G = np.concatenate(
        [mem_sm.T @ W_rv[:, k * M:(k + 1) * M].T for k in range(R)], axis=0
    )  # [512, 4H]

    wxT_h = np.ascontiguousarray(
        W_x.T.reshape(2, 128, 4 * H).transpose(1, 0, 2), dtype=np.float32
    )
    whhT_h = np.ascontiguousarray(W_hh.T, dtype=np.float32)
    import ml_dtypes
    gdt = ml_dtypes.bfloat16 if bf16_mix else np.float32
    gmat_h = np.ascontiguousarray(
        G.reshape(4, 128, 4 * H).transpose(1, 0, 2).astype(gdt)
    )
    biasb_h = np.ascontiguousarray(
        np.broadcast_to(bias.astype(np.float32), (128, 4 * H))
    )
    bias1b_h = np.ascontiguousarray(
        np.broadcast_to(bias1.astype(np.float32), (128, 4 * H))
    )

    in_maps = []
    for d in range(NCORES):
        xs = x[d * RL:(d + 1) * RL, :t_steps, :]          # [RL, t, 256]
        x2 = xs.transpose(1, 2, 0)                        # [t, 256, RL]
        xT_h = np.ascontiguousarray(
            x2.reshape(t_steps, 2, 128, RL).transpose(0, 2, 1, 3)
        )                                                 # [t, 128, 2, RL]
        in_maps.append(
            {
                "xT": xT_h,
                "wxT": wxT_h,
                "whhT": whhT_h,
                "gmat": gmat_h,
                "biasb": biasb_h,
                "bias1b": bias1b_h,
            }
        )
    return in_maps, mem_sm


def host_finish(inputs, hsum, t_steps=T):
    """Final fc layer + time-mean from hsum [B, H] (linear in hsum)."""
    memory = np.asarray(inputs["memory"], dtype=np.float64)
    fc_w = np.asarray(inputs["fc_w"], dtype=np.float64)
    fc_b = np.asarray(inputs["fc_b"], dtype=np.float64)

    mm = memory - memory.max(axis=0, keepdims=True)
    e = np.exp(mm)
    mem_sm = e / e.sum(axis=0, keepdims=True)

    fc_h = fc_w[:, :H]  # [OUT, H]
    Fstack = np.concatenate(
        [mem_sm.T @ fc_w[:, H + k * M:H + (k + 1) * M].T for k in range(R)],
        axis=0,
    )  # [512, OUT]

    hs = hsum.astype(np.float64)
    mixout = hs.reshape(B // 4, 4 * H) @ Fstack           # [256, OUT]
    out = (hs @ fc_h.T + mixout[np.arange(B) % (B // 4)]) / t_steps + fc_b
    return out.astype(np.float32)


BF16_MIX = False


def kernel(**inputs):
    from concourse.bass_utils import run_bass_kernel_spmd

    key = (T, BF16_MIX)
    if key not in _PROGRAM_CACHE:
        _PROGRAM_CACHE[key] = build_program(T, bf16_mix=BF16_MIX)
    nc = _PROGRAM_CACHE[key]

    in_maps, _ = host_prep(inputs, T, bf16_mix=BF16_MIX)
    res = run_bass_kernel_spmd(nc, in_maps, core_ids=list(range(NCORES)))
    hsum = np.concatenate(
        [res.results[d]["hsum_out"] for d in range(NCORES)], axis=0
    )  # [B, H]
    return host_finish(inputs, hsum, T)
